# revision 7
# baseline (speedup 1.0000x reference)
"""BERT self-attention on 8 Trainium2 NeuronCores (Bass/Tile).

Problem: B=8, S=1024, H=1024, NH=16, HD=64, fp32.
Sharding: pure data-parallel — one batch element per core, weights
replicated. No collectives.

Math notes:
- The attention-mask bias broadcasts over keys ((1-mask)[...,None] is a
  per-(batch,query) constant added to every logit of a softmax row), so
  it cancels exactly in softmax for any finite mask. It is not used.
- Softmax is computed without max-subtraction: logits are ~N(0,1)
  (|max| < ~6), exp is comfortably within fp32 range.
- All matmuls run in float32r (fp32 rounded to 13-bit mantissa by the
  producing op; full PE streaming rate at moving-dim >= 256).

Per-core pipeline (v2 — K-projection interleaved with attention so the
ACT-bound softmax exp overlaps PE matmul phases):
  X:  XT[h,s] = x^T                  (PE transposes, 4-per-PSUM-bank,
                                      batched copies alternate DVE/ACT)
  V:  Vpad[s,(h,d|1)] = [x@Wv^T+bv | 1]   (streamed per 256-col block)
  per o-tile ot (= head pair 2ot, 2ot+1):
    QT_ot[o,s] = Wq_ot @ x^T + bq    (streamed weight transpose)
    KT_ot[o,s] = Wk_ot @ x^T + bk
    per q-block: scoresT = KT^T-slice.T @ QT-slice (even head rows 0:64,
      odd head rows 64:128 — distinct PE row groups), E = exp(s/8),
      pv = [V|1]^T E (M=65), PE-transpose, ctx = pv[:,:64]/pv[:,64] + bv
"""
import numpy as np
from contextlib import ExitStack

import concourse.bass as bass
import concourse.tile as tile
from concourse import bacc, mybir
from concourse.bass_utils import run_bass_kernel_spmd
from concourse.masks import make_identity

B, S, H, NH = 8, 1024, 1024, 16
HD = H // NH          # 64
P = 128
NT = S // P           # 8 s-tiles
HT = H // P           # 8 h-tiles (contraction)
OT = H // P           # 8 o-tiles / head pairs
QBS = 512             # q-block size
NQB = S // QBS        # 2 q-blocks
N_CORES = 8
F32 = mybir.dt.float32
F32R = mybir.dt.float32r
AF = mybir.ActivationFunctionType
ALU = mybir.AluOpType

_CACHE = {}


def _emit(tc):
    nc = tc.nc
    x = nc.dram_tensor("x", [S, H], F32, kind="ExternalInput").ap()
    wq = nc.dram_tensor("wq", [H, H], F32, kind="ExternalInput").ap()
    wk = nc.dram_tensor("wk", [H, H], F32, kind="ExternalInput").ap()
    wv = nc.dram_tensor("wv", [H, H], F32, kind="ExternalInput").ap()
    bq = nc.dram_tensor("bq", [H], F32, kind="ExternalInput").ap()
    bk = nc.dram_tensor("bk", [H], F32, kind="ExternalInput").ap()
    bv = nc.dram_tensor("bv", [H], F32, kind="ExternalInput").ap()
    out = nc.dram_tensor("out", [S, H], F32, kind="ExternalOutput").ap()

    copy_flip = [0]

    def quad_copy(dst_ap, src_ap):
        # alternate PSUM->SBUF batched copies between DVE and ACT
        if copy_flip[0] % 2 == 0:
            nc.vector.tensor_copy(dst_ap, src_ap)
        else:
            nc.scalar.copy(dst_ap, src_ap)
        copy_flip[0] += 1

    with ExitStack() as top:
        consts = top.enter_context(tc.tile_pool(name="consts", bufs=1))
        nat = top.enter_context(tc.tile_pool(name="nat", bufs=3))
        big = top.enter_context(tc.tile_pool(name="big", bufs=1))
        wt = top.enter_context(tc.tile_pool(name="wt", bufs=2))
        qk = top.enter_context(tc.tile_pool(name="qk", bufs=2))
        cp = top.enter_context(tc.tile_pool(name="cp", bufs=2))
        ep = top.enter_context(tc.tile_pool(name="ep", bufs=1))

        ident = consts.tile([P, P], F32)
        make_identity(nc, ident[:])
        bq_sb = consts.tile([P, OT], F32, tag="bq")
        nc.sync.dma_start(bq_sb[:], bq.rearrange("(t p) -> p t", p=P))
        bk_sb = consts.tile([P, OT], F32, tag="bk")
        nc.sync.dma_start(bk_sb[:], bk.rearrange("(t p) -> p t", p=P))
        bv_row = consts.tile([1, H], F32, tag="bv_row")
        nc.sync.dma_start(bv_row[:], bv.unsqueeze(0))
        bv_bc = consts.tile([P, H], F32, tag="bv_bc")
        nc.gpsimd.partition_broadcast(bv_bc[:], bv_row[:])
        ones_f32 = consts.tile([P, P], F32, tag="ones")
        nc.vector.memset(ones_f32[:], 1.0)

        XT = big.tile([P, HT, S], F32R, tag="XT")    # XT[p, ht, s] = x[s, ht*P+p]
        Vpad = big.tile([P, NT, NH, HD + 1], F32R, tag="Vpad")
        stage = big.tile([P, NT, NH, HD], F32, tag="stage")

        # ---------------- phase X + V (own PSUM scope)
        with ExitStack() as pha:
            tra = pha.enter_context(tc.tile_pool(name="tra", bufs=3, space="PSUM"))
            vmm = pha.enter_context(tc.tile_pool(name="vmm", bufs=2, space="PSUM"))

            for st in range(NT):
                xn = nat.tile([P, H], F32, tag="nat")
                nc.sync.dma_start(xn[:], x.rearrange("(t p) h -> p t h", p=P)[:, st, :])
                for q2 in range(2):
                    tr = tra.tile([P, 4, P], F32, tag="tr")
                    for i in range(4):
                        ht = q2 * 4 + i
                        nc.tensor.transpose(tr[:, i, :], xn[:, ht * P:(ht + 1) * P],
                                            ident[:])
                    quad_copy(XT[:, q2 * 4:(q2 + 1) * 4, st * P:(st + 1) * P], tr[:])

            for otp in range(4):      # 256 output columns of V at a time
                wvT = wt.tile([P, HT, 2 * P], F32R, tag="wt")
                for n2 in range(2):
                    wn = nat.tile([P, H], F32, tag="nat")
                    nc.sync.dma_start(
                        wn[:],
                        wv.rearrange("(t p) h -> p t h", p=P)[:, otp * 2 + n2, :])
                    for q2 in range(2):
                        tr = tra.tile([P, 4, P], F32, tag="tr")
                        for i in range(4):
                            ht = q2 * 4 + i
                            nc.tensor.transpose(tr[:, i, :],
                                                wn[:, ht * P:(ht + 1) * P], ident[:])
                        quad_copy(wvT[:, q2 * 4:(q2 + 1) * 4, n2 * P:(n2 + 1) * P],
                                  tr[:])
                for stp in range(4):
                    vm = vmm.tile([P, 2, 2 * P], F32, tag="vm")
                    for j2 in range(2):
                        st = stp * 2 + j2
                        for ht in range(HT):
                            nc.tensor.matmul(
                                vm[:, j2, :], XT[:, ht, st * P:(st + 1) * P],
                                wvT[:, ht, :],
                                start=(ht == 0), stop=(ht == HT - 1))
                    for j2 in range(2):
                        st = stp * 2 + j2
                        nh0 = otp * 4   # 4 heads per 256-col block
                        nc.vector.tensor_tensor(
                            Vpad[:, st, nh0:nh0 + 4, 0:HD],
                            vm[:, j2, :].rearrange("p (h d) -> p h d", d=HD),
                            bv_bc[:, otp * 2 * P:(otp + 1) * 2 * P].rearrange(
                                "p (h d) -> p h d", d=HD),
                            ALU.add)
            # ones column of Vpad
            nc.vector.tensor_copy(
                Vpad[:, :, :, HD],
                ones_f32[:].rearrange("p (a b) -> p a b", a=NT))

        # ---------------- interleaved Q/K projection + attention per head pair
        with ExitStack() as phb:
            psb = phb.enter_context(tc.tile_pool(name="psb", bufs=2, space="PSUM"))

            def project_ot(w_ap, bias_sb, ot, out_tag):
                wT = wt.tile([P, HT, P], F32R, tag="wt")
                wn = nat.tile([P, H], F32, tag="nat")
                nc.sync.dma_start(
                    wn[:], w_ap.rearrange("(t p) h -> p t h", p=P)[:, ot, :])
                for q2 in range(2):
                    tr = psb.tile([P, 4, P], F32, tag="tr2")
                    for i in range(4):
                        ht = q2 * 4 + i
                        nc.tensor.transpose(tr[:, i, :], wn[:, ht * P:(ht + 1) * P],
                                            ident[:])
                    quad_copy(wT[:, q2 * 4:(q2 + 1) * 4, :], tr[:])
                acc = psb.tile([P, NQB, QBS], F32, tag="s")
                for sb in range(NQB):
                    for ht in range(HT):
                        nc.tensor.matmul(
                            acc[:, sb, :], wT[:, ht, :],
                            XT[:, ht, sb * QBS:(sb + 1) * QBS],
                            start=(ht == 0), stop=(ht == HT - 1))
                dst = qk.tile([P, S], F32R, tag=out_tag)
                nc.vector.tensor_scalar_add(
                    dst[:].rearrange("p (a b) -> p a b", a=NQB), acc[:],
                    bias_sb[:, ot:ot + 1])
                return dst

            for ot in range(OT):
                qt = project_ot(wq, bq_sb, ot, "qt")
                kt_ = project_ot(wk, bk_sb, ot, "kt")
                for qb in range(NQB):
                    E = ep.tile([P, NT, 2, QBS], F32R, tag="E")
                    for kt in range(NT):
                        ss = psb.tile([P, 2, QBS], F32, tag="s")
                        for j in range(2):
                            pr = slice(j * HD, (j + 1) * HD)
                            nc.tensor.matmul(
                                ss[:, j, :],
                                kt_[pr, kt * P:(kt + 1) * P],
                                qt[pr, qb * QBS:(qb + 1) * QBS],
                                start=True, stop=True)
                        nc.scalar.activation(E[:, kt, :, :], ss[:],
                                             AF.Exp, scale=0.125)
                    for j in range(2):
                        h = 2 * ot + j
                        pv = psb.tile([HD + 1, QBS], F32, tag="pv")
                        for kt in range(NT):
                            nc.tensor.matmul(
                                pv[:], Vpad[:, kt, h, :], E[:, kt, j, :],
                                start=(kt == 0), stop=(kt == NT - 1))
                        ctxT = cp.tile([HD + 1, QBS], F32, tag="ctxT")
                        nc.vector.tensor_copy(ctxT[:], pv[:])
                        trt = psb.tile([P, QBS // P, HD + 1], F32, tag="tr2")
                        for c in range(QBS // P):
                            nc.tensor.transpose(
                                trt[:, c, :], ctxT[:, c * P:(c + 1) * P],
                                ident[:HD + 1, :HD + 1])
                        rc = cp.tile([P, QBS // P], F32, tag="rc")
                        for c in range(QBS // P):
                            nc.vector.reciprocal(rc[:, c:c + 1], trt[:, c, HD:HD + 1])
                        for c in range(QBS // P):
                            st = qb * (QBS // P) + c
                            nc.vector.scalar_tensor_tensor(
                                stage[:, st, h, :], trt[:, c, 0:HD],
                                rc[:, c:c + 1],
                                bv_bc[:, h * HD:(h + 1) * HD],
                                ALU.mult, ALU.add)

            for st in range(NT):
                nc.sync.dma_start(
                    out.rearrange("(t p) o -> p t o", p=P)[:, st, :],
                    stage[:, st, :, :])


def build():
    if "nc" in _CACHE:
        return _CACHE["nc"]
    nc = bacc.Bacc("TRN2", target_bir_lowering=False, debug=False,
                   num_devices=N_CORES)
    with tile.TileContext(nc) as tc:
        _emit(tc)
    nc.compile()
    _CACHE["nc"] = nc
    return nc


def make_in_maps(hidden_state, Wq, bq, Wk, bk, Wv, bv):
    hs = np.ascontiguousarray(np.asarray(hidden_state, dtype=np.float32))
    common = {
        "wq": np.ascontiguousarray(np.asarray(Wq, np.float32)),
        "wk": np.ascontiguousarray(np.asarray(Wk, np.float32)),
        "wv": np.ascontiguousarray(np.asarray(Wv, np.float32)),
        "bq": np.ascontiguousarray(np.asarray(bq, np.float32)),
        "bk": np.ascontiguousarray(np.asarray(bk, np.float32)),
        "bv": np.ascontiguousarray(np.asarray(bv, np.float32)),
    }
    return [{"x": hs[i], **common} for i in range(N_CORES)]


def kernel(hidden_state, attention_mask, Wq, bq, Wk, bk, Wv, bv):
    # attention_mask: per-(batch, query) additive constant -> cancels in
    # softmax (see module docstring); unused.
    nc = build()
    in_maps = make_in_maps(hidden_state, Wq, bq, Wk, bk, Wv, bv)
    res = run_bass_kernel_spmd(nc, in_maps, list(range(N_CORES)))
    return np.stack([res.results[i]["out"] for i in range(N_CORES)], axis=0)


# revision 11
# speedup vs baseline: 1.2051x; 1.2051x over previous
"""BERT self-attention on 8 Trainium2 NeuronCores (Bass/Tile).

Problem: B=8, S=1024, H=1024, NH=16, HD=64, fp32.
Sharding: pure data-parallel — one batch element per core, weights
replicated. No collectives.

Math notes:
- The attention-mask bias broadcasts over keys ((1-mask)[...,None] is a
  per-(batch,query) constant added to every logit of a softmax row), so
  it cancels exactly in softmax for any finite mask. It is not used.
- Softmax is computed without max-subtraction: logits are ~N(0,1)
  (|max| < ~6), exp is comfortably within fp32 range.
- All matmuls run in float32r (fp32 rounded to 13-bit mantissa by the
  producing op; full PE streaming rate at moving-dim >= 256).

Per-core pipeline (v2 — K-projection interleaved with attention so the
ACT-bound softmax exp overlaps PE matmul phases):
  X:  XT[h,s] = x^T                  (PE transposes, 4-per-PSUM-bank,
                                      batched copies alternate DVE/ACT)
  V:  Vpad[s,(h,d|1)] = [x@Wv^T+bv | 1]   (streamed per 256-col block)
  per o-tile ot (= head pair 2ot, 2ot+1):
    QT_ot[o,s] = Wq_ot @ x^T + bq    (streamed weight transpose)
    KT_ot[o,s] = Wk_ot @ x^T + bk
    per q-block: scoresT = KT^T-slice.T @ QT-slice (even head rows 0:64,
      odd head rows 64:128 — distinct PE row groups), E = exp(s/8),
      pv = [V|1]^T E (M=65), PE-transpose, ctx = pv[:,:64]/pv[:,64] + bv
"""
import numpy as np
from contextlib import ExitStack

import concourse.bass as bass
import concourse.tile as tile
from concourse import bacc, mybir
from concourse.bass_utils import run_bass_kernel_spmd
from concourse.masks import make_identity

B, S, H, NH = 8, 1024, 1024, 16
HD = H // NH          # 64
P = 128
NT = S // P           # 8 s-tiles
HT = H // P           # 8 h-tiles (contraction)
OT = H // P           # 8 o-tiles / head pairs
QBS = 512             # q-block size
NQB = S // QBS        # 2 q-blocks
N_CORES = 8
F32 = mybir.dt.float32
F32R = mybir.dt.float32r
AF = mybir.ActivationFunctionType
ALU = mybir.AluOpType

_CACHE = {}


def _emit(tc):
    nc = tc.nc
    x = nc.dram_tensor("x", [S, H], F32, kind="ExternalInput").ap()
    wq = nc.dram_tensor("wq", [H, H], F32, kind="ExternalInput").ap()
    wk = nc.dram_tensor("wk", [H, H], F32, kind="ExternalInput").ap()
    wv = nc.dram_tensor("wv", [H, H], F32, kind="ExternalInput").ap()
    bq = nc.dram_tensor("bq", [H], F32, kind="ExternalInput").ap()
    bk = nc.dram_tensor("bk", [H], F32, kind="ExternalInput").ap()
    bv = nc.dram_tensor("bv", [H], F32, kind="ExternalInput").ap()
    out = nc.dram_tensor("out", [S, H], F32, kind="ExternalOutput").ap()

    copy_flip = [0]

    def quad_copy(dst_ap, src_ap):
        # alternate PSUM->SBUF batched copies between DVE and ACT
        if copy_flip[0] % 2 == 0:
            nc.vector.tensor_copy(dst_ap, src_ap)
        else:
            nc.scalar.copy(dst_ap, src_ap)
        copy_flip[0] += 1

    with ExitStack() as top:
        consts = top.enter_context(tc.tile_pool(name="consts", bufs=1))
        nat = top.enter_context(tc.tile_pool(name="nat", bufs=3))
        big = top.enter_context(tc.tile_pool(name="big", bufs=1))
        wt = top.enter_context(tc.tile_pool(name="wt", bufs=2))
        qk = top.enter_context(tc.tile_pool(name="qk", bufs=2))
        cp = top.enter_context(tc.tile_pool(name="cp", bufs=2))
        ep = top.enter_context(tc.tile_pool(name="ep", bufs=2))

        ident = consts.tile([P, P], F32)
        make_identity(nc, ident[:])
        bq_sb = consts.tile([P, OT], F32, tag="bq")
        nc.sync.dma_start(bq_sb[:], bq.rearrange("(t p) -> p t", p=P))
        bk_sb = consts.tile([P, OT], F32, tag="bk")
        nc.sync.dma_start(bk_sb[:], bk.rearrange("(t p) -> p t", p=P))
        bv_row = consts.tile([1, H], F32, tag="bv_row")
        nc.sync.dma_start(bv_row[:], bv.unsqueeze(0))
        bv_bc = consts.tile([P, H], F32, tag="bv_bc")
        nc.gpsimd.partition_broadcast(bv_bc[:], bv_row[:])
        ones_f32 = consts.tile([P, P], F32, tag="ones")
        nc.vector.memset(ones_f32[:], 1.0)

        XT = big.tile([P, HT, S], F32R, tag="XT")    # XT[p, ht, s] = x[s, ht*P+p]
        Vpad = big.tile([P, NT, NH, HD + 1], F32R, tag="Vpad")

        # ---------------- phase X + V (own PSUM scope)
        with ExitStack() as pha:
            tra = pha.enter_context(tc.tile_pool(name="tra", bufs=3, space="PSUM"))
            vmm = pha.enter_context(tc.tile_pool(name="vmm", bufs=2, space="PSUM"))

            for st in range(NT):
                xn = nat.tile([P, H], F32, tag="nat")
                nc.sync.dma_start(xn[:], x.rearrange("(t p) h -> p t h", p=P)[:, st, :])
                for q2 in range(2):
                    tr = tra.tile([P, 4, P], F32, tag="tr")
                    for i in range(4):
                        ht = q2 * 4 + i
                        nc.tensor.transpose(tr[:, i, :], xn[:, ht * P:(ht + 1) * P],
                                            ident[:])
                    quad_copy(XT[:, q2 * 4:(q2 + 1) * 4, st * P:(st + 1) * P], tr[:])

            for otp in range(4):      # 256 output columns of V at a time
                wvT = wt.tile([P, HT, 2 * P], F32R, tag="wt")
                for n2 in range(2):
                    wn = nat.tile([P, H], F32, tag="nat")
                    nc.sync.dma_start(
                        wn[:],
                        wv.rearrange("(t p) h -> p t h", p=P)[:, otp * 2 + n2, :])
                    for q2 in range(2):
                        tr = tra.tile([P, 4, P], F32, tag="tr")
                        for i in range(4):
                            ht = q2 * 4 + i
                            nc.tensor.transpose(tr[:, i, :],
                                                wn[:, ht * P:(ht + 1) * P], ident[:])
                        quad_copy(wvT[:, q2 * 4:(q2 + 1) * 4, n2 * P:(n2 + 1) * P],
                                  tr[:])
                for stp in range(4):
                    vm = vmm.tile([P, 2, 2 * P], F32, tag="vm")
                    for j2 in range(2):
                        st = stp * 2 + j2
                        for ht in range(HT):
                            nc.tensor.matmul(
                                vm[:, j2, :], XT[:, ht, st * P:(st + 1) * P],
                                wvT[:, ht, :],
                                start=(ht == 0), stop=(ht == HT - 1))
                    for j2 in range(2):
                        st = stp * 2 + j2
                        nh0 = otp * 4   # 4 heads per 256-col block
                        nc.vector.tensor_tensor(
                            Vpad[:, st, nh0:nh0 + 4, 0:HD],
                            vm[:, j2, :].rearrange("p (h d) -> p h d", d=HD),
                            bv_bc[:, otp * 2 * P:(otp + 1) * 2 * P].rearrange(
                                "p (h d) -> p h d", d=HD),
                            ALU.add)
            # ones column of Vpad
            nc.vector.tensor_copy(
                Vpad[:, :, :, HD],
                ones_f32[:].rearrange("p (a b) -> p a b", a=NT))

        # ---------------- interleaved Q/K projection + attention per head pair
        # Software pipeline: per ot emit [project Q/K(ot), PV(ot-1), scores(ot)]
        # so the ACT-bound exp(ot-1) overlaps the PE-bound projections of ot.
        with ExitStack() as phb:
            psb = phb.enter_context(tc.tile_pool(name="psb", bufs=2, space="PSUM"))
            ctp = phb.enter_context(tc.tile_pool(name="ctp", bufs=4))

            def project_ot(w_ap, bias_sb, ot, out_tag):
                wT = wt.tile([P, HT, P], F32R, tag="wt")
                wn = nat.tile([P, H], F32, tag="nat")
                nc.sync.dma_start(
                    wn[:], w_ap.rearrange("(t p) h -> p t h", p=P)[:, ot, :])
                for q2 in range(2):
                    tr = psb.tile([P, 4, P], F32, tag="tr2")
                    for i in range(4):
                        ht = q2 * 4 + i
                        nc.tensor.transpose(tr[:, i, :], wn[:, ht * P:(ht + 1) * P],
                                            ident[:])
                    quad_copy(wT[:, q2 * 4:(q2 + 1) * 4, :], tr[:])
                acc = psb.tile([P, NQB, QBS], F32, tag="s")
                for sb in range(NQB):
                    for ht in range(HT):
                        nc.tensor.matmul(
                            acc[:, sb, :], wT[:, ht, :],
                            XT[:, ht, sb * QBS:(sb + 1) * QBS],
                            start=(ht == 0), stop=(ht == HT - 1))
                dst = qk.tile([P, S], F32R, tag=out_tag)
                nc.vector.tensor_scalar_add(
                    dst[:].rearrange("p (a b) -> p a b", a=NQB), acc[:],
                    bias_sb[:, ot:ot + 1])
                return dst

            out_tiled = out.rearrange("(t p) o -> p t o", p=P)

            def emit_scores(ot, qt, kt_):
                Es = []
                for qb in range(NQB):
                    E = ep.tile([P, NT, 2, QBS], F32R, tag="E")
                    for kt in range(NT):
                        ss = psb.tile([P, 2, QBS], F32, tag="s")
                        for j in range(2):
                            pr = slice(j * HD, (j + 1) * HD)
                            nc.tensor.matmul(
                                ss[:, j, :],
                                kt_[pr, kt * P:(kt + 1) * P],
                                qt[pr, qb * QBS:(qb + 1) * QBS],
                                start=True, stop=True)
                        nc.scalar.activation(E[:, kt, :, :], ss[:],
                                             AF.Exp, scale=0.125)
                    Es.append(E)
                return Es

            def emit_pv(ot, Es):
                for qb in range(NQB):
                    E = Es[qb]
                    for j in range(2):
                        h = 2 * ot + j
                        pv = psb.tile([HD + 1, QBS], F32, tag="pv")
                        for kt in range(NT):
                            nc.tensor.matmul(
                                pv[:], Vpad[:, kt, h, :], E[:, kt, j, :],
                                start=(kt == 0), stop=(kt == NT - 1))
                        ctxT = cp.tile([HD + 1, QBS], F32, tag="ctxT")
                        nc.vector.tensor_copy(ctxT[:], pv[:])
                        trt = psb.tile([P, QBS // P, HD + 1], F32, tag="tr2")
                        for c in range(QBS // P):
                            nc.tensor.transpose(
                                trt[:, c, :], ctxT[:, c * P:(c + 1) * P],
                                ident[:HD + 1, :HD + 1])
                        rc = cp.tile([P, QBS // P], F32, tag="rc")
                        for c in range(QBS // P):
                            nc.vector.reciprocal(rc[:, c:c + 1], trt[:, c, HD:HD + 1])
                        for c in range(QBS // P):
                            st = qb * (QBS // P) + c
                            ct = ctp.tile([P, HD], F32, tag="ct")
                            nc.vector.scalar_tensor_tensor(
                                ct[:], trt[:, c, 0:HD], rc[:, c:c + 1],
                                bv_bc[:, h * HD:(h + 1) * HD],
                                ALU.mult, ALU.add)
                            nc.sync.dma_start(
                                out_tiled[:, st, h * HD:(h + 1) * HD], ct[:])

            pending = None
            for ot in range(OT):
                qt = project_ot(wq, bq_sb, ot, "qt")
                kt_ = project_ot(wk, bk_sb, ot, "kt")
                if pending is not None:
                    emit_pv(*pending)
                pending = (ot, emit_scores(ot, qt, kt_))
            emit_pv(*pending)


def build():
    if "nc" in _CACHE:
        return _CACHE["nc"]
    nc = bacc.Bacc("TRN2", target_bir_lowering=False, debug=False,
                   num_devices=N_CORES)
    with tile.TileContext(nc) as tc:
        _emit(tc)
    nc.compile()
    _CACHE["nc"] = nc
    return nc


def make_in_maps(hidden_state, Wq, bq, Wk, bk, Wv, bv):
    hs = np.ascontiguousarray(np.asarray(hidden_state, dtype=np.float32))
    common = {
        "wq": np.ascontiguousarray(np.asarray(Wq, np.float32)),
        "wk": np.ascontiguousarray(np.asarray(Wk, np.float32)),
        "wv": np.ascontiguousarray(np.asarray(Wv, np.float32)),
        "bq": np.ascontiguousarray(np.asarray(bq, np.float32)),
        "bk": np.ascontiguousarray(np.asarray(bk, np.float32)),
        "bv": np.ascontiguousarray(np.asarray(bv, np.float32)),
    }
    return [{"x": hs[i], **common} for i in range(N_CORES)]


def kernel(hidden_state, attention_mask, Wq, bq, Wk, bk, Wv, bv):
    # attention_mask: per-(batch, query) additive constant -> cancels in
    # softmax (see module docstring); unused.
    nc = build()
    in_maps = make_in_maps(hidden_state, Wq, bq, Wk, bk, Wv, bv)
    res = run_bass_kernel_spmd(nc, in_maps, list(range(N_CORES)))
    return np.stack([res.results[i]["out"] for i in range(N_CORES)], axis=0)
